# revision 22
# baseline (speedup 1.0000x reference)
"""Trainium2 Bass kernel for the BoW language model head problem.

Model (per reference):
    emb = wte[x] + wpe            (B,T,C)
    h   = emb + cumsum(emb)/[1..T]
    h   = h + tanh(h@w_fc+b_fc)@w_proj + b_proj
    out = h @ w_head + b_head     (B,T,V)

Shapes: B=4, T=2048, V=50257, C=512, H=2048.

Sharding (8 cores): core i owns batch i//2 and token half i%2 (1024
tokens), and computes the full vocab for those tokens.  No collectives:
the only cross-half coupling is the causal-BoW prefix base, which each
second-half core computes locally from a gather of the first half's
embeddings (first-half cores compute it too, masked to zero — SPMD).

The BoW runs C-major: for each 128-token block j,
    pbT_c = e_j^T @ [trid_j | 1]          (4 matmuls, N=129)
gives the in-block weighted prefix sums transposed (C on partitions)
plus the block sum in the last column.  The cross-block running sum
rides as a per-partition bias on ScalarE (tmp = pbT + S; next S is
tmp[:,128]), and the final h^T block is tmp * recipT on VectorE.

Scheduling notes (hard-won):
  - embeddings use dma_gather (1024 rows per instruction, ~1.3us of
    gpsimd time) instead of per-block INDIRECT1D (~1.1us per 128 rows).
    dma_gather indices are int16 and the ucode skips "negative" ones,
    so each gather is a lo/hi pair: idx<32768 from wte, idx>=32768
    (as -1 in the lo pass) from wte[32768:], merged by position into
    the same tile.
  - every dma_start costs ~0.7us of dispatch on its sequencer, and a
    DMA consumer can wait on *all* earlier DMAs of the same queue —
    so the gather indices go first on the scalar ring, consts are
    batched, and w_head group loads ride the gpsimd ring (idle after
    the gathers) so their dispatch never blocks PSUM-drain copies.
  - Tile dependency tracking is whole-tile: hT/hF/wfc/wpe are split
    into half tiles and the MLP proj PSUM uses one tile per C-chunk,
    otherwise consumers stall on writes they don't read.
  - a burst of identity transposes at t=0 keeps the PE HAM clock warm
    while the gathers land.
"""

from contextlib import ExitStack

import ml_dtypes
import numpy as np

import concourse.bacc as bacc
import concourse.bass as bass
import concourse.mybir as mybir
import concourse.tile as tile
from concourse.bass_utils import run_bass_kernel_spmd

P = 128
B, T, V, C, H = 4, 2048, 50257, 512, 2048
NBLK = 16              # token blocks per batch
NLOC = 8               # local token blocks per core
TLOC = NLOC * P        # 1024 local tokens
NCC = C // P           # 4 C chunks
NHC = H // P           # 16 H chunks
TG = 512               # token group width (MLP moving dim)
VT = 512               # vocab tile width
NT = 99                # vocab tiles (98 full + 1 tail)
TAIL_W = V - 98 * VT   # 81 useful columns in the last vocab tile
VGROUPS = [(g * 8, 8) for g in range(12)] + [(96, 3)]
WARMUP_TP = 64         # identity transposes to keep the PE clock warm
VSP = 32767            # lo/hi split: idx 32767 stays free for a zero row
NHI = V - VSP          # 17490 hi rows; idx NHI is the hi zero row

# batched-constant layouts (columns within cb16 / cf32)
CB16_W = NLOC * (P + 1) + P          # trid_aug (8*129) + ident (128)
IDENT_O = NLOC * (P + 1)
CF32_W = NLOC * P + NCC + 1 + NHC + NCC
WSUM_O = NLOC * P
MASK_O = WSUM_O + NCC
BFC_O = MASK_O + 1
BPROJ_O = BFC_O + NHC
# gather index columns within idxg [P, IDXG_W]: oth(lo,hi) 64+64,
# own_a(lo,hi) 32+32, own_b(lo,hi) 32+32
IDXG_W = 256

F32 = mybir.dt.float32
BF16 = mybir.dt.bfloat16
I16 = mybir.dt.int16

BF = ml_dtypes.bfloat16


def _build_nc():
    nc = bacc.Bacc(None, target_bir_lowering=False, debug=True,
                   num_swdge_queues=4, num_devices=8)

    idxg = nc.dram_tensor("idxg", [P, IDXG_W], I16, kind="ExternalInput")
    # wte2 = [wte[0:VSP]; zeros; wte[VSP:]; zeros] — dma_gather indices
    # are int16, so tokens split lo/hi at VSP; "invalid" entries point at
    # a zero row INSIDE their slice (trailing-negative indices would be
    # silently skipped, leaving stale SBUF), and lo+hi merge by addition.
    wte2 = nc.dram_tensor("wte2", [V + 2, C], BF16, kind="ExternalInput")
    wpe_own = nc.dram_tensor("wpe_own", [TLOC, C], BF16,
                             kind="ExternalInput")
    w_fc = nc.dram_tensor("w_fc", [C, H], BF16, kind="ExternalInput")
    w_proj = nc.dram_tensor("w_proj", [H, C], BF16, kind="ExternalInput")
    w_head = nc.dram_tensor("w_head", [P, NT, NCC, VT], BF16,
                            kind="ExternalInput")
    cb16 = nc.dram_tensor("cb16", [P, CB16_W], BF16, kind="ExternalInput")
    cf32 = nc.dram_tensor("cf32", [P, CF32_W], F32, kind="ExternalInput")
    out = nc.dram_tensor("out", [TLOC, V], BF16, kind="ExternalOutput")

    wpe_r = wpe_own.rearrange("(k p) c -> p k c", p=P)

    with tile.TileContext(nc) as tc:
        stack_bc = ExitStack()
        with tc.tile_pool(name="consts", bufs=1) as consts, \
             tc.tile_pool(name="whp", bufs=3) as whp, \
             tc.tile_pool(name="hfp", bufs=1) as hfp:
            htp = stack_bc.enter_context(tc.tile_pool(name="htp", bufs=1))
            wmats = stack_bc.enter_context(tc.tile_pool(name="wmats", bufs=1))

            # gather indices FIRST on the scalar ring (nothing may precede).
            idxg_sb = consts.tile([P, IDXG_W], I16, tag="idxg")
            nc.scalar.dma_start(out=idxg_sb[:], in_=idxg[:])

            # batched consts + wpe halves on sync
            cb16_sb = consts.tile([P, CB16_W], BF16, tag="cb16")
            nc.sync.dma_start(out=cb16_sb[:], in_=cb16[:])
            cf32_sb = consts.tile([P, CF32_W], F32, tag="cf32")
            nc.sync.dma_start(out=cf32_sb[:], in_=cf32[:])
            w_own = [consts.tile([P, NLOC // 2, C], BF16, tag=f"wpe{i}",
                                 name="wpe") for i in range(2)]
            nc.sync.dma_start(out=w_own[0][:], in_=wpe_r[:, 0:4, :])
            nc.sync.dma_start(out=w_own[1][:], in_=wpe_r[:, 4:8, :])
            sbase_sb = consts.tile([P, NCC], F32, tag="sbase")

            def trid_ap(j):
                return cb16_sb[:, j * (P + 1):(j + 1) * (P + 1)]

            ident_ap = cb16_sb[:, IDENT_O:IDENT_O + P]
            ones_col = cb16_sb[:, P:P + 1]  # trid_0 col 128 == 1.0

            def recipT_ap(j):
                return cf32_sb[:, j * P:(j + 1) * P]

            # wfc halves early on the scalar ring (first needed by MLP g0)
            wfc_sb = [wmats.tile([P, NCC, H // 2], BF16, tag=f"wfc{i}",
                                 name="wfc") for i in range(2)]
            nc.scalar.dma_start(out=wfc_sb[0][:],
                                in_=w_fc.rearrange("(c p) h -> p c h",
                                                   p=P)[:, :, 0:H // 2])
            nc.scalar.dma_start(out=wfc_sb[1][:],
                                in_=w_fc.rearrange("(c p) h -> p c h",
                                                   p=P)[:, :, H // 2:H])
            wproj_sb = wmats.tile([P, NHC, C], BF16, tag="wproj")

            def load_whg(gi, eng):
                v0, nv = VGROUPS[gi]
                wh = whp.tile([P, nv, NCC, VT], BF16, tag="whg")
                eng.dma_start(out=wh[:], in_=w_head[:, v0:v0 + nv, :, :])
                return wh

            hT = [htp.tile([P, NCC, TG], BF16, tag=f"hT{i}", name="hT")
                  for i in range(2)]
            hF = [hfp.tile([P, NCC, TG], BF16, tag=f"hF{i}", name="hF")
                  for i in range(2)]

            # burn the HAM cold window while the gathers land
            with tc.tile_pool(name="warm", bufs=1, space="PSUM") as warm:
                wt = warm.tile([P, P], BF16, tag="warm")
                for _ in range(WARMUP_TP):
                    nc.tensor.transpose(wt[:], ident_ap, ident_ap)

            # ---------------- Phase B: embedding + causal BoW ----------------
            with tc.tile_pool(name="embp", bufs=1) as embp, \
                 tc.tile_pool(name="ep", bufs=4) as ep, \
                 tc.tile_pool(name="tmpp", bufs=10) as tmpp, \
                 tc.tile_pool(name="psO", bufs=1, space="PSUM") as psO, \
                 tc.tile_pool(name="psb", bufs=5, space="PSUM") as psb:
                g_oth = [embp.tile([P, NLOC, C], BF16, tag=f"g_oth{i}",
                                   name="g_oth") for i in range(2)]
                g_own = [embp.tile([P, NLOC // 2, C], BF16, tag=f"g_own{i}",
                                   name="g_own") for i in range(4)]

                # lo/hi gather pairs into separate tiles (invalid entries
                # gather the zero row); the other half first (it gates the
                # prefix base O -> the whole ScalarE bias chain).
                def gather_pair(dst_lo, dst_hi, col0, ni):
                    nw = ni // 16
                    nc.gpsimd.dma_gather(
                        dst_lo[:], wte2[0:VSP + 1, :],
                        idxg_sb[:, col0:col0 + nw], ni, ni, C, elem_step=C)
                    nc.gpsimd.dma_gather(
                        dst_hi[:], wte2[VSP + 1:V + 2, :],
                        idxg_sb[:, col0 + nw:col0 + 2 * nw],
                        ni, ni, C, elem_step=C)

                gather_pair(g_oth[0], g_oth[1], 0, TLOC)
                gather_pair(g_own[0], g_own[1], 128, TLOC // 2)
                gather_pair(g_own[2], g_own[3], 192, TLOC // 2)

                # deferred big loads, serialized behind the gathers on the
                # gpsimd engine stream (wproj needed ~45us, whg0 ~90us)
                nc.gpsimd.dma_start(
                    out=wproj_sb[:],
                    in_=w_proj.rearrange("(hc p) c -> p hc c", p=P))
                whgs_pre = [load_whg(gi, nc.gpsimd) for gi in range(3)]

                # O (other-half embedding sum), C-major via N=1 matmuls into
                # 4-byte PSUM accumulation regions (one bank total).
                psO4 = psO.tile([P, NCC], F32, tag="O")
                for c in range(NCC):
                    for k in range(2 * NLOC):
                        nc.tensor.matmul(
                            psO4[:, c:c + 1],
                            lhsT=g_oth[k // NLOC][:, k % NLOC,
                                                  c * P:(c + 1) * P],
                            rhs=ones_col,
                            start=(k == 0), stop=(k == 2 * NLOC - 1))
                for c in range(NCC):
                    nc.scalar.activation(
                        sbase_sb[:, c:c + 1], psO4[:, c:c + 1],
                        mybir.ActivationFunctionType.Identity,
                        bias=cf32_sb[:, WSUM_O + c:WSUM_O + c + 1],
                        scale=cf32_sb[:, MASK_O:MASK_O + 1])

                # own half: BoW matmuls (PE), bias chain (ACT), scale (DVE)
                e_tiles = [None] * NLOC

                def emit_e(j):
                    gl = g_own[2 * (j // 4)][:, j % 4, :]
                    gh = g_own[2 * (j // 4) + 1][:, j % 4, :]
                    t_j = ep.tile([P, C], BF16, tag="E2", name="t_j")
                    nc.vector.tensor_add(t_j[:], gl, gh)
                    e_j = ep.tile([P, C], BF16, tag="E", name="e_j")
                    nc.vector.tensor_add(e_j[:], t_j[:],
                                         w_own[j // 4][:, j % 4, :])
                    e_tiles[j] = e_j

                emit_e(0)
                emit_e(1)
                prev_tmp = {}
                for j in range(NLOC):
                    ps_bow = {}
                    for c in range(NCC):
                        pb = psb.tile([P, P + 1], F32, tag="bow", name="pb")
                        nc.tensor.matmul(pb[:],
                                         lhsT=e_tiles[j][:, c * P:(c + 1) * P],
                                         rhs=trid_ap(j),
                                         start=True, stop=True)
                        ps_bow[c] = pb
                    cur_tmp = {}
                    for c in range(NCC):
                        bias_ap = (sbase_sb[:, c:c + 1] if j == 0
                                   else prev_tmp[c][:, P:P + 1])
                        tmp = tmpp.tile([P, P + 1], F32, tag="tmp", name="tmp")
                        nc.scalar.activation(
                            tmp[:], ps_bow[c][:],
                            mybir.ActivationFunctionType.Identity,
                            bias=bias_ap)
                        nc.vector.tensor_tensor(
                            out=hT[j // 4][:, c, (j % 4) * P:(j % 4 + 1) * P],
                            in0=tmp[:, 0:P], in1=recipT_ap(j),
                            op=mybir.AluOpType.mult)
                        cur_tmp[c] = tmp
                    prev_tmp = cur_tmp
                    if j + 2 < NLOC:
                        emit_e(j + 2)

            # ---------------- Phase C: MLP ----------------
            with tc.tile_pool(name="ap_", bufs=NHC) as ap_, \
                 tc.tile_pool(name="ctmp", bufs=3) as ctmp, \
                 tc.tile_pool(name="psfc", bufs=2, space="PSUM") as psfc, \
                 tc.tile_pool(name="pspj", bufs=4, space="PSUM") as pspj:
                for gidx in range(TLOC // TG):
                    a_tiles = []
                    for hc in range(NHC):
                        pfc = psfc.tile([P, TG], F32, tag="fc")
                        wfc_h = wfc_sb[hc // 8]
                        hco = (hc % 8) * P
                        for c in range(NCC):
                            nc.tensor.matmul(
                                pfc[:], lhsT=wfc_h[:, c, hco:hco + P],
                                rhs=hT[gidx][:, c, :],
                                start=(c == 0), stop=(c == NCC - 1))
                        a = ap_.tile([P, TG], BF16, tag="a")
                        nc.scalar.activation(a[:], pfc[:],
                                             mybir.ActivationFunctionType.Tanh,
                                             bias=cf32_sb[:, BFC_O + hc:BFC_O + hc + 1])
                        a_tiles.append(a)
                    for cc in range(NCC):
                        ppj = pspj.tile([P, TG], F32, tag="proj", name="ppj")
                        for hc in range(NHC):
                            nc.tensor.matmul(
                                ppj[:],
                                lhsT=wproj_sb[:, hc, cc * P:(cc + 1) * P],
                                rhs=a_tiles[hc][:],
                                start=(hc == 0), stop=(hc == NHC - 1))
                        tmpc = ctmp.tile([P, TG], BF16, tag="tmpc")
                        nc.scalar.activation(tmpc[:], ppj[:],
                                             mybir.ActivationFunctionType.Identity,
                                             bias=cf32_sb[:, BPROJ_O + cc:BPROJ_O + cc + 1])
                        nc.vector.tensor_add(hF[gidx][:, cc, :], tmpc[:],
                                             hT[gidx][:, cc, :])

            # ---------------- Phase D: head ----------------
            stack_bc.close()  # free wfc/wproj + hT SBUF for the head phase
            with tc.tile_pool(name="stp", bufs=6) as stp, \
                 tc.tile_pool(name="pso", bufs=8, space="PSUM") as pso:
                copy_ctr = [0]

                def head_block(j, whg, v0, nv):
                    jo = (j % 4) * P
                    src = hF[j // 4]
                    for q0 in range(0, nv, 4):
                        qn = min(4, nv - q0)
                        ws = [TAIL_W if v0 + q0 + qi == NT - 1 else VT
                              for qi in range(qn)]
                        psq = [pso.tile([P, ws[qi]], F32, tag="po", name="po")
                               for qi in range(qn)]
                        for c in range(NCC):
                            for qi in range(qn):
                                nc.tensor.matmul(
                                    psq[qi][:], lhsT=src[:, c, jo:jo + P],
                                    rhs=whg[:, q0 + qi, c, :ws[qi]],
                                    start=(c == 0), stop=(c == NCC - 1))
                        st = stp.tile([P, 4 * VT], BF16, tag="stage")
                        o0 = 0
                        for qi in range(qn):
                            dst = st[:, o0:o0 + ws[qi]]
                            if copy_ctr[0] % 2:
                                nc.scalar.activation(
                                    dst, psq[qi][:],
                                    mybir.ActivationFunctionType.Copy)
                            else:
                                nc.vector.tensor_copy(dst, psq[qi][:])
                            copy_ctr[0] += 1
                            o0 += ws[qi]
                        nc.sync.dma_start(
                            out=out[j * P:(j + 1) * P,
                                    (v0 + q0) * VT:(v0 + q0) * VT + o0],
                            in_=st[:, :o0])

                whgs = list(whgs_pre)
                for gi in range(len(VGROUPS)):
                    whg = whgs[gi]
                    v0, nv = VGROUPS[gi]
                    for j in range(NLOC):
                        head_block(j, whg, v0, nv)
                    if gi + 3 < len(VGROUPS):
                        # gpsimd ring (idle in phase D): the dispatch cost
                        # never lands on the ACT/DVE/SYNC streams
                        whgs.append(load_whg(gi + 3, nc.gpsimd))
    nc.compile()
    return nc


_NC = None


def _get_nc():
    global _NC
    if _NC is None:
        _NC = _build_nc()
    return _NC


def _wrap_idx(tokens, lo):
    """Pack a token list into the dma_gather idx layout [P, len//16] int16.

    lo=True keeps idx < VHI (others -> -1); lo=False emits idx-VHI for
    idx >= VHI (others -> -1).  idx i lives at [i%16, i//16], replicated
    across the 8 groups of 16 partitions.
    """
    n = len(tokens)
    # invalid marker: a VALID index pointing at the zero row inside the
    # slice (negative indices at the tail would be skipped -> stale SBUF)
    if lo:
        v = np.where(tokens < VSP, tokens, VSP).astype(np.uint16)
    else:
        v = np.where(tokens >= VSP, tokens - VSP, NHI).astype(np.uint16)
    w = np.zeros((16, n // 16), np.uint16)
    w[np.arange(n) % 16, np.arange(n) // 16] = v
    return np.tile(w, (8, 1)).view(np.int16)


def make_in_maps(x, wte, wpe, w_fc, b_fc, w_proj, b_proj, w_head, b_head):
    x = np.asarray(x).astype(np.int64)
    wte_b = np.asarray(wte, np.float32).astype(BF)
    wte2_b = np.zeros((V + 2, C), BF)
    wte2_b[0:VSP] = wte_b[:VSP]         # lo slice; row VSP stays zero
    wte2_b[VSP + 1:V + 1] = wte_b[VSP:]  # hi slice; row V+1 stays zero
    wpe_b = np.asarray(wpe, np.float32).astype(BF)
    wfc_b = np.asarray(w_fc, np.float32).astype(BF)
    wproj_b = np.asarray(w_proj, np.float32).astype(BF)
    whead_b = np.asarray(w_head, np.float32).astype(BF)
    b_fc = np.asarray(b_fc, dtype=np.float32)
    b_proj = np.asarray(b_proj, dtype=np.float32)

    # [C, V] -> [P, NT, NCC, VT] with the tail tile zero-padded
    pad = np.zeros((C, NT * VT), BF)
    pad[:, :V] = whead_b
    wh_packed = np.ascontiguousarray(
        pad.reshape(NCC, P, NT, VT).transpose(1, 2, 0, 3))

    b_fc2d = b_fc.reshape(NHC, P).T            # [P, NHC]
    b_proj2d = b_proj.reshape(NCC, P).T        # [P, NCC]
    ident_np = np.eye(P, dtype=np.float32)

    cb16s, cf32s = [], []
    for vh in range(2):
        cb = np.zeros((P, CB16_W), np.float32)
        for j in range(NLOC):
            t = np.triu(np.ones((P, P), np.float32))
            base = (vh * NLOC + j) * P
            # h = recip*(prefix_excl + diag*e) must equal e + bow, so
            # diag = denom + 1; col 128 = 1 gives the block sum.
            t[np.arange(P), np.arange(P)] = base + np.arange(P) + 2
            cb[:, j * (P + 1):(j + 1) * (P + 1) - 1] = t
            cb[:, (j + 1) * (P + 1) - 1] = 1.0
        cb[:, IDENT_O:IDENT_O + P] = ident_np
        cb16s.append(np.ascontiguousarray(cb.astype(BF)))

        cf = np.zeros((P, CF32_W), np.float32)
        tglob = vh * TLOC + np.arange(TLOC, dtype=np.float32).reshape(NLOC, P)
        cf[:, :NLOC * P] = (1.0 / (tglob + 1.0)).reshape(1, -1)
        oth = wpe_b[(1 - vh) * TLOC:(2 - vh) * TLOC].astype(np.float32)
        cf[:, WSUM_O:WSUM_O + NCC] = float(vh) * oth.sum(axis=0).reshape(NCC, P).T
        cf[:, MASK_O] = float(vh)
        cf[:, BFC_O:BFC_O + NHC] = b_fc2d
        cf[:, BPROJ_O:BPROJ_O + NCC] = b_proj2d
        cf32s.append(np.ascontiguousarray(cf))

    in_maps = []
    for core in range(8):
        b = core // 2
        vh = core % 2
        own = x[b, vh * TLOC:(vh + 1) * TLOC]
        oth = x[b, (1 - vh) * TLOC:(2 - vh) * TLOC]
        ig = np.concatenate([
            _wrap_idx(oth, True), _wrap_idx(oth, False),
            _wrap_idx(own[:TG], True), _wrap_idx(own[:TG], False),
            _wrap_idx(own[TG:], True), _wrap_idx(own[TG:], False),
        ], axis=1)
        in_maps.append({
            "idxg": np.ascontiguousarray(ig),
            "wte2": wte2_b,
            "wpe_own": np.ascontiguousarray(
                wpe_b[vh * TLOC:(vh + 1) * TLOC]),
            "w_fc": wfc_b,
            "w_proj": wproj_b,
            "w_head": wh_packed,
            "cb16": cb16s[vh],
            "cf32": cf32s[vh],
        })
    return in_maps


def kernel(x, wte, wpe, w_fc, b_fc, w_proj, b_proj, w_head, b_head):
    b_head = np.asarray(b_head, dtype=np.float32)
    in_maps = make_in_maps(x, wte, wpe, w_fc, b_fc, w_proj, b_proj,
                           w_head, b_head)
    nc = _get_nc()
    res = run_bass_kernel_spmd(nc, in_maps, core_ids=list(range(8)))

    logits = np.empty((B, T, V), np.float32)
    for core in range(8):
        b = core // 2
        vh = core % 2
        logits[b, vh * TLOC:(vh + 1) * TLOC, :] = \
            res.results[core]["out"].astype(np.float32)
    if b_head.any():
        logits += b_head[None, None, :]
    return logits
